# revision 21
# baseline (speedup 1.0000x reference)
"""Multi-head attention (B=2, S=2048, D=1024, H=16, dk=64) on 8 trn2 cores.

Sharding: core c handles batch b=c//4 and 4 heads g=c%4 (heads 4g..4g+3).
Each core computes its heads' Q/K/V projections, attention, and a partial
output projection; the host sums the 4 partials per batch.

Per-core kernel layout:
  - host pre-transposes x -> xT [D, S] so projections contract over D.
  - all three x tensors stream up-front (3 SBUF buffers) with chunk DMAs
    fanned across the gpsimd/SP/Activation queues; projections chase the
    arriving chunks.
  - qhT/khT [128 = 2 heads x 64 d, S] (bf16), per head-pair hp.
  - scores are computed directly TRANSPOSED: scT[k 128-chunk, q 512] =
    (khT chunk).T @ qhT — the exp output feeds the PV matmul with no
    transpose. Two k-chunks share a [128,1024] PSUM tile; one ScalarE
    exp converts both to bf16 SBUF.
  - softmax has no max-subtraction (scores ~ N(0,1), safe in fp32/bf16).
    The denominator is free: vh gets a 65th all-ones column, so the PV
    matmul ctx_aug[65, q] = vh_aug.T @ expT accumulates ctx rows 0..63
    and the exp row-sum in row 64.
  - attention is software-pipelined per head: each head's 16 score
    matmuls + 8 exps interleave with the PREVIOUS head's 16 PV matmuls
    and one output-projection S-tile, so ScalarE (the pacing engine)
    never starves.
  - normalization happens on the tiny ctx tile, all lane-aligned: the
    denom row is broadcast over partitions on GpSimd (software op), the
    reciprocal runs on the broadcast [64,512] tile on DVE, and one fused
    multiply-cast writes ctxT (bf16). Odd heads reach partitions 64..127
    via an SBUF staging tile + DMA (engines cannot cross partitions).
  - bk/bv/bo never touch the device: bk only shifts scores by a per-row
    constant (softmax-invariant), bv/bo fold into a host-side correction
    bv @ wo.T + bo added once per batch. bq is added on-device (scaled by
    1/sqrt(dk), which is folded into wq/bq on the host).
"""

import sys

for _p in ("/opt/trn_rl_repo",):
    if _p not in sys.path:
        sys.path.insert(0, _p)

from contextlib import ExitStack

import ml_dtypes
import numpy as np

import concourse.bass as bass
import concourse.bacc as bacc_mod
import concourse.mybir as mybir
import concourse.tile as tile
from concourse.bass_utils import run_bass_kernel_spmd

BF16 = mybir.dt.bfloat16
F32 = mybir.dt.float32
AF = mybir.ActivationFunctionType

B, S, D = 2, 2048, 1024
NCORES = 8
HLOC = 4          # heads per core
DK = 64
HD = HLOC * DK    # local head dims = 256
KT_D = D // 128   # 8 k-tiles over the model dim
NST = S // 128    # 16 tiles over sequence
NQG = 4           # q groups of 512
NKC = S // 128    # 16 k chunks of 128 for scores/PV


def build_nc() -> bass.Bass:
    nc = bacc_mod.Bacc()

    xqT = nc.dram_tensor("xqT", [D, S], BF16, kind="ExternalInput")
    xkT = nc.dram_tensor("xkT", [D, S], BF16, kind="ExternalInput")
    xvT = nc.dram_tensor("xvT", [D, S], BF16, kind="ExternalInput")
    wqT = nc.dram_tensor("wqT", [D, HD], BF16, kind="ExternalInput")
    wkT = nc.dram_tensor("wkT", [D, HD], BF16, kind="ExternalInput")
    wvT = nc.dram_tensor("wvT", [D, HD], BF16, kind="ExternalInput")
    woT = nc.dram_tensor("woT", [HD, D], BF16, kind="ExternalInput")
    bqd = nc.dram_tensor("bq", [HD], BF16, kind="ExternalInput")
    out = nc.dram_tensor("out_partial", [S, D], F32, kind="ExternalOutput")

    with tile.TileContext(nc) as tc, ExitStack() as ctx:
        const = ctx.enter_context(tc.tile_pool(name="const", bufs=1))
        persist = ctx.enter_context(tc.tile_pool(name="persist", bufs=1))

        # Weights / bias resident in SBUF. wq/bq/wk load first (needed
        # earliest); wv/wo are triggered after the x loads.
        wq_s = const.tile([128, KT_D, HD], BF16, tag="wq")
        wk_s = const.tile([128, KT_D, HD], BF16, tag="wk")
        wv_s = const.tile([128, KT_D, HD], BF16, tag="wv")
        wo_s = const.tile([128, 2, D], BF16, tag="wo")
        bq_s = const.tile([1, HD], BF16, tag="bq")
        ones_row = const.tile([1, S], BF16, tag="ones")
        nc.vector.memset(ones_row, 1.0)
        # fp32 ones column used by the PE rank-1 broadcast of the softmax
        # reciprocal row (row 64 = the lane the denominator lives on)
        ones_col = const.tile([128, DK], F32, tag="onesc")
        nc.vector.memset(ones_col, 1.0)
        nc.sync.dma_start(wq_s, wqT.rearrange("(t p) n -> p t n", p=128))
        nc.sync.dma_start(bq_s, bqd.rearrange("(o n) -> o n", o=1))
        nc.sync.dma_start(wk_s, wkT.rearrange("(t p) n -> p t n", p=128))

        # Projection outputs resident in SBUF.
        qhT = [persist.tile([128, S], BF16, tag=f"qhT{m}", name=f"qhT{m}") for m in range(2)]
        khT = [persist.tile([128, S], BF16, tag=f"khT{m}", name=f"khT{m}") for m in range(2)]
        # vh with a 65th all-ones column per (head, k-chunk): PV's 65th
        # output row becomes the softmax denominator.
        vh_aug = persist.tile([128, HLOC, NST, DK + 1], BF16, tag="vh")
        nc.vector.memset(vh_aug[:, :, :, DK : DK + 1], 1.0)
        ctxT = [persist.tile([128, S], BF16, tag=f"ctxT{m}", name=f"ctxT{m}") for m in range(2)]

        # ---- Phase 1: projections -------------------------------------
        # x chunk DMAs fan out over the three DMA-capable queues (gpsimd
        # swdge, SP hwdge, Activation hwdge) and all three tensors load
        # up-front into 3 buffers — phase 1 is input-bandwidth-bound.
        dma_engines = [nc.gpsimd, nc.sync, nc.scalar]

        def load_x(dst, src):
            for _kt in range(KT_D):
                dma_engines[_kt % 3].dma_start(
                    dst[:, _kt], src[_kt * 128 : (_kt + 1) * 128, :]
                )

        with tc.tile_pool(name="xload", bufs=3) as xpool:
          with tc.tile_pool(name="qk_psum", bufs=2, space="PSUM") as qkp:
            xq_t = xpool.tile([128, KT_D, S], BF16, tag="x", name="xq_t")
            xk_t = xpool.tile([128, KT_D, S], BF16, tag="x", name="xk_t")
            xv_t = xpool.tile([128, KT_D, S], BF16, tag="x", name="xv_t")
            load_x(xq_t, xqT)
            load_x(xk_t, xkT)
            load_x(xv_t, xvT)
            nc.sync.dma_start(wv_s, wvT.rearrange("(t p) n -> p t n", p=128))
            nc.sync.dma_start(wo_s, woT.rearrange("(t p) n -> p t n", p=128))

            for m in range(2):
                ps = qkp.tile([128, S], F32, tag="proj")
                for kt in range(KT_D):
                    for nb in range(4):
                        nc.tensor.matmul(
                            ps[:, nb * 512 : (nb + 1) * 512],
                            lhsT=wq_s[:, kt, m * 128 : (m + 1) * 128],
                            rhs=xq_t[:, kt, nb * 512 : (nb + 1) * 512],
                            start=(kt == 0),
                            stop=False,
                        )
                # bias as rank-1 matmul: qhT[p, n] += bq[p] * 1
                for nb in range(4):
                    nc.tensor.matmul(
                        ps[:, nb * 512 : (nb + 1) * 512],
                        lhsT=bq_s[0:1, m * 128 : (m + 1) * 128],
                        rhs=ones_row[0:1, nb * 512 : (nb + 1) * 512],
                        start=False,
                        stop=True,
                    )
                nc.vector.tensor_copy(qhT[m], ps)

            for m in range(2):
                ps = qkp.tile([128, S], F32, tag="proj")
                for kt in range(KT_D):
                    for nb in range(4):
                        nc.tensor.matmul(
                            ps[:, nb * 512 : (nb + 1) * 512],
                            lhsT=wk_s[:, kt, m * 128 : (m + 1) * 128],
                            rhs=xk_t[:, kt, nb * 512 : (nb + 1) * 512],
                            start=(kt == 0),
                            stop=(kt == KT_D - 1),
                        )
                nc.vector.tensor_copy(khT[m], ps)

          with tc.tile_pool(name="v_psum", bufs=4, space="PSUM") as vps:
                for st in range(NST):
                    ps = vps.tile([128, HD], F32, tag="vproj")
                    for kt in range(KT_D):
                        nc.tensor.matmul(
                            ps,
                            lhsT=xv_t[:, kt, st * 128 : (st + 1) * 128],
                            rhs=wv_s[:, kt, :],
                            start=(kt == 0),
                            stop=(kt == KT_D - 1),
                        )
                    # scatter the 4 heads' [128, 64] blocks into the
                    # 65-strided vh_aug layout (ones column untouched)
                    nc.vector.tensor_copy(vh_aug[:, :, st, 0:DK], ps)

        # ---- Phase 2+3: software-pipelined attention + out-proj -------
        with (
            tc.tile_pool(name="sc_psum", bufs=2, space="PSUM") as scp,
            tc.tile_pool(name="ctx_psum", bufs=3, space="PSUM") as ctxp,
            tc.tile_pool(name="bc_psum", bufs=1, space="PSUM") as bcpp,
            tc.tile_pool(name="exp_pool", bufs=3) as epool,
            tc.tile_pool(name="stat_pool", bufs=2) as stp,
            tc.tile_pool(name="bc_pool", bufs=2) as bcp,
            tc.tile_pool(name="ctxs_pool", bufs=2) as ctxsp,
            tc.tile_pool(name="out_sbuf", bufs=3) as outs,
        ):
            def emit_outproj_st(st):
                ops = scp.tile([128, 1024], F32, tag="sc", name="ops")
                for nb in range(2):
                    for kt in range(2):
                        nc.tensor.matmul(
                            ops[:, nb * 512 : (nb + 1) * 512],
                            lhsT=ctxT[kt][:, st * 128 : (st + 1) * 128],
                            rhs=wo_s[:, kt, nb * 512 : (nb + 1) * 512],
                            start=(kt == 0),
                            stop=(kt == 1),
                        )
                ob = outs.tile([128, D], F32, tag="ob")
                nc.vector.tensor_copy(ob, ops)
                nc.sync.dma_start(out[st * 128 : (st + 1) * 128, :], ob)

            def emit_pv_pair(stage, kp):
                _qg, h_, exp_, ctx_ = stage[:4]
                for j in range(2):
                    kt = kp * 2 + j
                    nc.tensor.matmul(
                        ctx_[0 : DK + 1, :],
                        lhsT=vh_aug[:, h_, kt, :],
                        rhs=exp_[:, kp, j * 512 : (j + 1) * 512],
                        start=(kt == 0),
                        stop=(kt == NKC - 1),
                    )

            F32R = mybir.dt.float32r

            def emit_recip(stage):
                # native in-lane reciprocal of the denom row (custom DVE
                # ops are broken at a partition offset; the native op is
                # exact there). Runs a full stage before its consumer so
                # the PE never waits on it.
                den = stage[4]
                nc.vector.reciprocal(den[DK : DK + 1, :], stage[3][DK : DK + 1, :])

            def emit_bcmul(stage):
                qg_, h_, _exp, ctx_, den = stage
                hp_, e_ = divmod(h_, 2)
                q0_ = qg_ * 512
                # PE broadcasts the reciprocal row to lanes 0..63 via a
                # rank-1 ones-column matmul (fp32r: 1 cycle/row), then one
                # fused multiply-cast writes ctxT.
                bc_ps = bcpp.tile([DK, 512], F32, tag="bcp", name="bc_ps")
                nc.tensor.matmul(
                    bc_ps,
                    lhsT=ones_col[DK : DK + 1, :],
                    rhs=den[DK : DK + 1, :],
                    start=True,
                    stop=True,
                )
                rec = bcp.tile([DK, 512], F32, tag="rec", name="rec")
                nc.vector.tensor_copy(rec, bc_ps)
                if e_ == 0:
                    nc.vector.tensor_mul(
                        ctxT[hp_][0:DK, q0_ : q0_ + 512], ctx_[0:DK, :], rec
                    )
                else:
                    ctxs = ctxsp.tile([DK, 512], BF16, tag="ctxs", name="ctxs")
                    nc.vector.tensor_mul(ctxs, ctx_[0:DK, :], rec)
                    nc.sync.dma_start(
                        ctxT[hp_][DK : 2 * DK, q0_ : q0_ + 512], ctxs
                    )

            stages = [(qg, h) for qg in range(NQG) for h in range(HLOC)]
            pending: list[int] = []
            prev = None   # stage i-1: its PV matmuls run during this stage
            prev2 = None  # stage i-2: its broadcast+multiply run now
            for qg, h in stages:
                hp, e = divmod(h, 2)
                po, q0 = e * 64, qg * 512
                exp_t = epool.tile([128, NKC // 2, 1024], BF16, tag="exp", name="exp_t")
                ctx_ps = ctxp.tile([128, 512], F32, tag="ctx", name="ctx_ps")
                if prev2 is not None:
                    emit_bcmul(prev2)
                    if prev2[1] == HLOC - 1:
                        pending += [prev2[0] * 4 + k for k in range(4)]
                for kp in range(NKC // 2):
                    sc = scp.tile([128, 1024], F32, tag="sc", name="sc")
                    for j in range(2):
                        kt = kp * 2 + j
                        nc.tensor.matmul(
                            sc[:, j * 512 : (j + 1) * 512],
                            lhsT=khT[hp][po : po + 64, kt * 128 : (kt + 1) * 128],
                            rhs=qhT[hp][po : po + 64, q0 : q0 + 512],
                            start=True,
                            stop=True,
                        )
                    nc.scalar.activation(exp_t[:, kp, :], sc, AF.Exp)
                    if prev is not None:
                        emit_pv_pair(prev, kp)
                    if kp == 4 and pending:
                        emit_outproj_st(pending.pop(0))
                    if kp == 6 and len(pending) >= 3:
                        emit_outproj_st(pending.pop(0))
                if prev is not None:
                    emit_recip(prev)
                den_t = stp.tile([128, 512], F32, tag="den", name="den_t")
                prev2 = prev
                prev = (qg, h, exp_t, ctx_ps, den_t)
            # drain: stage 14's bcmul, stage 15's PV + recip + bcmul, then
            # the remaining out-proj tiles
            emit_bcmul(prev2)
            for kp in range(NKC // 2):
                emit_pv_pair(prev, kp)
            emit_recip(prev)
            emit_bcmul(prev)
            pending += [(NQG - 1) * 4 + k for k in range(4)]
            for st in pending:
                emit_outproj_st(st)

    nc.compile()
    return nc


_CACHE: dict = {}


def _get_nc() -> bass.Bass:
    if "nc" not in _CACHE:
        _CACHE["nc"] = build_nc()
    return _CACHE["nc"]


def _bf16(x: np.ndarray) -> np.ndarray:
    return np.ascontiguousarray(x).astype(ml_dtypes.bfloat16)


def make_in_maps(q, k, v, wq, bq, wk, bk, wv, bv, wo, bo):
    scale = np.float32(1.0 / np.sqrt(DK))
    in_maps = []
    for c in range(NCORES):
        b, g = divmod(c, 4)
        hh = g * HD
        in_maps.append(
            {
                "xqT": _bf16(np.asarray(q[b], np.float32).T),
                "xkT": _bf16(np.asarray(k[b], np.float32).T),
                "xvT": _bf16(np.asarray(v[b], np.float32).T),
                "wqT": _bf16(np.asarray(wq[hh : hh + HD], np.float32).T * scale),
                "wkT": _bf16(np.asarray(wk[hh : hh + HD], np.float32).T),
                "wvT": _bf16(np.asarray(wv[hh : hh + HD], np.float32).T),
                "woT": _bf16(np.asarray(wo[:, hh : hh + HD], np.float32).T),
                "bq": _bf16(np.asarray(bq[hh : hh + HD], np.float32) * scale),
            }
        )
    return in_maps


def assemble(results, bv, bo, wo) -> np.ndarray:
    out = np.zeros((B, S, D), np.float32)
    for c in range(NCORES):
        out[c // 4] += np.asarray(results[c]["out_partial"], np.float32)
    corr = np.asarray(bv, np.float32) @ np.asarray(wo, np.float32).T + np.asarray(
        bo, np.float32
    )
    out += corr[None, None, :]
    return out


def kernel(q, k, v, wq, bq, wk, bk, wv, bv, wo, bo) -> np.ndarray:
    nc = _get_nc()
    in_maps = make_in_maps(q, k, v, wq, bq, wk, bk, wv, bv, wo, bo)
    res = run_bass_kernel_spmd(nc, in_maps, list(range(NCORES))).results
    return assemble(res, bv, bo, wo)


# revision 25
# speedup vs baseline: 1.2044x; 1.2044x over previous
"""Multi-head attention (B=2, S=2048, D=1024, H=16, dk=64) on 8 trn2 cores.

Sharding: core c handles batch b=c//4 and 4 heads g=c%4 (heads 4g..4g+3).
Each core computes its heads' Q/K/V projections, attention, and a partial
output projection; the host sums the 4 partials per batch.

Per-core kernel layout:
  - host pre-transposes x -> xT [D, S] so projections contract over D.
  - all three x tensors stream up-front (3 SBUF buffers) with chunk DMAs
    fanned across the gpsimd/SP/Activation queues; projections chase the
    arriving chunks.
  - qhT/khT [128 = 2 heads x 64 d, S] (bf16), per head-pair hp.
  - scores are computed directly TRANSPOSED: scT[k 128-chunk, q 512] =
    (khT chunk).T @ qhT — the exp output feeds the PV matmul with no
    transpose. Two k-chunks share a [128,1024] PSUM tile; one ScalarE
    exp converts both to bf16 SBUF.
  - softmax has no max-subtraction (scores ~ N(0,1), safe in fp32/bf16).
    The denominator is free: vh gets a 65th all-ones column, so the PV
    matmul ctx_aug[65, q] = vh_aug.T @ expT accumulates ctx rows 0..63
    and the exp row-sum in row 64.
  - attention is software-pipelined per head: each head's 16 score
    matmuls + 8 exps interleave with the PREVIOUS head's 16 PV matmuls
    and one output-projection S-tile, so ScalarE (the pacing engine)
    never starves.
  - normalization happens on the tiny ctx tile, all lane-aligned: the
    denom row is broadcast over partitions on GpSimd (software op), the
    reciprocal runs on the broadcast [64,512] tile on DVE, and one fused
    multiply-cast writes ctxT (bf16). Odd heads reach partitions 64..127
    via an SBUF staging tile + DMA (engines cannot cross partitions).
  - bk/bv/bo never touch the device: bk only shifts scores by a per-row
    constant (softmax-invariant), bv/bo fold into a host-side correction
    bv @ wo.T + bo added once per batch. bq is added on-device (scaled by
    1/sqrt(dk), which is folded into wq/bq on the host).
"""

import sys

for _p in ("/opt/trn_rl_repo",):
    if _p not in sys.path:
        sys.path.insert(0, _p)

from contextlib import ExitStack

import ml_dtypes
import numpy as np

import concourse.bass as bass
import concourse.bacc as bacc_mod
import concourse.mybir as mybir
import concourse.tile as tile
from concourse.bass_utils import run_bass_kernel_spmd

BF16 = mybir.dt.bfloat16
F32 = mybir.dt.float32
AF = mybir.ActivationFunctionType

B, S, D = 2, 2048, 1024
NCORES = 8
HLOC = 4          # heads per core
DK = 64
HD = HLOC * DK    # local head dims = 256
KT_D = D // 128   # 8 k-tiles over the model dim
NST = S // 128    # 16 tiles over sequence
NQG = 4           # q groups of 512
NKC = S // 128    # 16 k chunks of 128 for scores/PV


def build_nc() -> bass.Bass:
    nc = bacc_mod.Bacc()

    xqT = nc.dram_tensor("xqT", [D, S], BF16, kind="ExternalInput")
    xkT = nc.dram_tensor("xkT", [D, S], BF16, kind="ExternalInput")
    xvT = nc.dram_tensor("xvT", [D, S], BF16, kind="ExternalInput")
    wqT = nc.dram_tensor("wqT", [D, HD], BF16, kind="ExternalInput")
    wkT = nc.dram_tensor("wkT", [D, HD], BF16, kind="ExternalInput")
    wvT = nc.dram_tensor("wvT", [D, HD], BF16, kind="ExternalInput")
    woT = nc.dram_tensor("woT", [HD, D], BF16, kind="ExternalInput")
    bqd = nc.dram_tensor("bq", [HD], BF16, kind="ExternalInput")
    out = nc.dram_tensor("out_partial", [S, D], F32, kind="ExternalOutput")

    with tile.TileContext(nc) as tc, ExitStack() as ctx:
        const = ctx.enter_context(tc.tile_pool(name="const", bufs=1))
        persist = ctx.enter_context(tc.tile_pool(name="persist", bufs=1))

        # Weights / bias resident in SBUF. wq/bq/wk load first (needed
        # earliest); wv/wo are triggered after the x loads.
        wq_s = const.tile([128, KT_D, HD], BF16, tag="wq")
        wk_s = const.tile([128, KT_D, HD], BF16, tag="wk")
        wv_s = const.tile([128, KT_D, HD], BF16, tag="wv")
        wo_s = const.tile([128, 2, D], BF16, tag="wo")
        bq_s = const.tile([1, HD], BF16, tag="bq")
        ones_row = const.tile([1, S], BF16, tag="ones")
        nc.vector.memset(ones_row, 1.0)
        # bf16 ones column used by the PE rank-1 broadcast of the softmax
        # reciprocal row (row 64 = the lane the denominator lives on)
        ones_col = const.tile([128, DK], BF16, tag="onesc")
        nc.vector.memset(ones_col, 1.0)
        nc.sync.dma_start(wq_s, wqT.rearrange("(t p) n -> p t n", p=128))
        nc.sync.dma_start(bq_s, bqd.rearrange("(o n) -> o n", o=1))
        nc.sync.dma_start(wk_s, wkT.rearrange("(t p) n -> p t n", p=128))

        # Projection outputs resident in SBUF.
        qhT = [persist.tile([128, S], BF16, tag=f"qhT{m}", name=f"qhT{m}") for m in range(2)]
        khT = [persist.tile([128, S], BF16, tag=f"khT{m}", name=f"khT{m}") for m in range(2)]
        # vh with a 65th all-ones column per (head, k-chunk): PV's 65th
        # output row becomes the softmax denominator.
        vh_aug = persist.tile([128, HLOC, NST, DK + 1], BF16, tag="vh")
        nc.vector.memset(vh_aug[:, :, :, DK : DK + 1], 1.0)
        ctxT = [persist.tile([128, S], BF16, tag=f"ctxT{m}", name=f"ctxT{m}") for m in range(2)]

        # ---- Phase 1: projections -------------------------------------
        # x chunk DMAs fan out over the three DMA-capable queues (gpsimd
        # swdge, SP hwdge, Activation hwdge) and all three tensors load
        # up-front into 3 buffers — phase 1 is input-bandwidth-bound.
        dma_engines = [nc.gpsimd, nc.sync, nc.scalar]

        def load_x(dst, src):
            for _kt in range(KT_D):
                dma_engines[_kt % 3].dma_start(
                    dst[:, _kt], src[_kt * 128 : (_kt + 1) * 128, :]
                )

        with tc.tile_pool(name="xload", bufs=3) as xpool:
          with tc.tile_pool(name="qk_psum", bufs=2, space="PSUM") as qkp:
            xq_t = xpool.tile([128, KT_D, S], BF16, tag="x", name="xq_t")
            xk_t = xpool.tile([128, KT_D, S], BF16, tag="x", name="xk_t")
            xv_t = xpool.tile([128, KT_D, S], BF16, tag="x", name="xv_t")
            load_x(xq_t, xqT)
            load_x(xk_t, xkT)
            load_x(xv_t, xvT)
            nc.sync.dma_start(wv_s, wvT.rearrange("(t p) n -> p t n", p=128))
            nc.sync.dma_start(wo_s, woT.rearrange("(t p) n -> p t n", p=128))

            for m in range(2):
                ps = qkp.tile([128, S], F32, tag="proj")
                for kt in range(KT_D):
                    for nb in range(4):
                        nc.tensor.matmul(
                            ps[:, nb * 512 : (nb + 1) * 512],
                            lhsT=wq_s[:, kt, m * 128 : (m + 1) * 128],
                            rhs=xq_t[:, kt, nb * 512 : (nb + 1) * 512],
                            start=(kt == 0),
                            stop=False,
                        )
                # bias as rank-1 matmul: qhT[p, n] += bq[p] * 1
                for nb in range(4):
                    nc.tensor.matmul(
                        ps[:, nb * 512 : (nb + 1) * 512],
                        lhsT=bq_s[0:1, m * 128 : (m + 1) * 128],
                        rhs=ones_row[0:1, nb * 512 : (nb + 1) * 512],
                        start=False,
                        stop=True,
                    )
                nc.vector.tensor_copy(qhT[m], ps)

            for m in range(2):
                ps = qkp.tile([128, S], F32, tag="proj")
                for kt in range(KT_D):
                    for nb in range(4):
                        nc.tensor.matmul(
                            ps[:, nb * 512 : (nb + 1) * 512],
                            lhsT=wk_s[:, kt, m * 128 : (m + 1) * 128],
                            rhs=xk_t[:, kt, nb * 512 : (nb + 1) * 512],
                            start=(kt == 0),
                            stop=(kt == KT_D - 1),
                        )
                nc.vector.tensor_copy(khT[m], ps)

          with tc.tile_pool(name="v_psum", bufs=4, space="PSUM") as vps:
                for st in range(NST):
                    ps = vps.tile([128, HD], F32, tag="vproj")
                    for kt in range(KT_D):
                        nc.tensor.matmul(
                            ps,
                            lhsT=xv_t[:, kt, st * 128 : (st + 1) * 128],
                            rhs=wv_s[:, kt, :],
                            start=(kt == 0),
                            stop=(kt == KT_D - 1),
                        )
                    # scatter the 4 heads' [128, 64] blocks into the
                    # 65-strided vh_aug layout (ones column untouched)
                    nc.vector.tensor_copy(vh_aug[:, :, st, 0:DK], ps)

        # ---- Phase 2+3: software-pipelined attention + out-proj -------
        with (
            tc.tile_pool(name="sc_psum", bufs=2, space="PSUM") as scp,
            tc.tile_pool(name="ctx_psum", bufs=2, space="PSUM") as ctxp,
            tc.tile_pool(name="bc_psum", bufs=2, space="PSUM") as bcpp,
            tc.tile_pool(name="exp_pool", bufs=3) as epool,
            tc.tile_pool(name="stat_pool", bufs=2) as stp,
            tc.tile_pool(name="bc_pool", bufs=2) as bcp,
            tc.tile_pool(name="ctxs_pool", bufs=2) as ctxsp,
            tc.tile_pool(name="out_sbuf", bufs=3) as outs,
        ):
            def emit_outproj_st(st):
                ops = scp.tile([128, 1024], F32, tag="sc", name="ops")
                for nb in range(2):
                    for kt in range(2):
                        nc.tensor.matmul(
                            ops[:, nb * 512 : (nb + 1) * 512],
                            lhsT=ctxT[kt][:, st * 128 : (st + 1) * 128],
                            rhs=wo_s[:, kt, nb * 512 : (nb + 1) * 512],
                            start=(kt == 0),
                            stop=(kt == 1),
                        )
                ob = outs.tile([128, D], F32, tag="ob")
                nc.vector.tensor_copy(ob, ops)
                nc.sync.dma_start(out[st * 128 : (st + 1) * 128, :], ob)

            def emit_pv_pair(stage, kp):
                _qg, h_, exp_, ctx_ = stage[:4]
                for j in range(2):
                    kt = kp * 2 + j
                    nc.tensor.matmul(
                        ctx_[0 : DK + 1, :],
                        lhsT=vh_aug[:, h_, kt, :],
                        rhs=exp_[:, kp, j * 512 : (j + 1) * 512],
                        start=(kt == 0),
                        stop=(kt == NKC - 1),
                    )

            def emit_norm(stage):
                qg_, h_, _exp, ctx_ = stage[:4]
                hp_, e_ = divmod(h_, 2)
                q0_ = qg_ * 512
                # native reciprocal of the denom row in-lane (custom DVE
                # ops are broken at a partition offset, the native op is
                # exact there), cast to bf16 in-lane, then the PE
                # broadcasts the row to lanes 0..63 via a rank-1
                # ones-column bf16 matmul (no PE dtype-mode switch), and
                # one fused multiply-cast writes ctxT.
                den = stp.tile([128, 512], F32, tag="den", name="den")
                nc.vector.reciprocal(den[DK : DK + 1, :], ctx_[DK : DK + 1, :])
                den16 = stp.tile([128, 512], BF16, tag="den16", name="den16")
                nc.vector.tensor_copy(den16[DK : DK + 1, :], den[DK : DK + 1, :])
                bc_ps = bcpp.tile([DK, 512], F32, tag="bcp", name="bc_ps")
                nc.tensor.matmul(
                    bc_ps,
                    lhsT=ones_col[DK : DK + 1, :],
                    rhs=den16[DK : DK + 1, :],
                    start=True,
                    stop=True,
                )
                rec = bcp.tile([DK, 512], F32, tag="rec", name="rec")
                nc.vector.tensor_copy(rec, bc_ps)
                if e_ == 0:
                    nc.vector.tensor_mul(
                        ctxT[hp_][0:DK, q0_ : q0_ + 512], ctx_[0:DK, :], rec
                    )
                else:
                    ctxs = ctxsp.tile([DK, 512], BF16, tag="ctxs", name="ctxs")
                    nc.vector.tensor_mul(ctxs, ctx_[0:DK, :], rec)
                    nc.sync.dma_start(
                        ctxT[hp_][DK : 2 * DK, q0_ : q0_ + 512], ctxs
                    )

            stages = [(qg, h) for qg in range(NQG) for h in range(HLOC)]
            pending: list[int] = []
            prev = None   # stage i-1: its PV matmuls + norm run during this stage
            for qg, h in stages:
                hp, e = divmod(h, 2)
                po, q0 = e * 64, qg * 512
                exp_t = epool.tile([128, NKC // 2, 1024], BF16, tag="exp", name="exp_t")
                ctx_ps = ctxp.tile([128, 512], F32, tag="ctx", name="ctx_ps")
                for kp in range(NKC // 2):
                    sc = scp.tile([128, 1024], F32, tag="sc", name="sc")
                    for j in range(2):
                        kt = kp * 2 + j
                        nc.tensor.matmul(
                            sc[:, j * 512 : (j + 1) * 512],
                            lhsT=khT[hp][po : po + 64, kt * 128 : (kt + 1) * 128],
                            rhs=qhT[hp][po : po + 64, q0 : q0 + 512],
                            start=True,
                            stop=True,
                        )
                    nc.scalar.activation(exp_t[:, kp, :], sc, AF.Exp)
                    if prev is not None:
                        emit_pv_pair(prev, kp)
                    if kp == 4 and pending:
                        emit_outproj_st(pending.pop(0))
                    if kp == 6 and len(pending) >= 3:
                        emit_outproj_st(pending.pop(0))
                if prev is not None:
                    emit_norm(prev)
                    if prev[1] == HLOC - 1:
                        pending += [prev[0] * 4 + k for k in range(4)]
                prev = (qg, h, exp_t, ctx_ps)
            # drain: last head's PV + norm, then remaining out-proj tiles
            for kp in range(NKC // 2):
                emit_pv_pair(prev, kp)
            emit_norm(prev)
            pending += [(NQG - 1) * 4 + k for k in range(4)]
            for st in pending:
                emit_outproj_st(st)

    nc.compile()
    return nc


_CACHE: dict = {}


def _get_nc() -> bass.Bass:
    if "nc" not in _CACHE:
        _CACHE["nc"] = build_nc()
    return _CACHE["nc"]


def _bf16(x: np.ndarray) -> np.ndarray:
    return np.ascontiguousarray(x).astype(ml_dtypes.bfloat16)


def make_in_maps(q, k, v, wq, bq, wk, bk, wv, bv, wo, bo):
    scale = np.float32(1.0 / np.sqrt(DK))
    in_maps = []
    for c in range(NCORES):
        b, g = divmod(c, 4)
        hh = g * HD
        in_maps.append(
            {
                "xqT": _bf16(np.asarray(q[b], np.float32).T),
                "xkT": _bf16(np.asarray(k[b], np.float32).T),
                "xvT": _bf16(np.asarray(v[b], np.float32).T),
                "wqT": _bf16(np.asarray(wq[hh : hh + HD], np.float32).T * scale),
                "wkT": _bf16(np.asarray(wk[hh : hh + HD], np.float32).T),
                "wvT": _bf16(np.asarray(wv[hh : hh + HD], np.float32).T),
                "woT": _bf16(np.asarray(wo[:, hh : hh + HD], np.float32).T),
                "bq": _bf16(np.asarray(bq[hh : hh + HD], np.float32) * scale),
            }
        )
    return in_maps


def assemble(results, bv, bo, wo) -> np.ndarray:
    out = np.zeros((B, S, D), np.float32)
    for c in range(NCORES):
        out[c // 4] += np.asarray(results[c]["out_partial"], np.float32)
    corr = np.asarray(bv, np.float32) @ np.asarray(wo, np.float32).T + np.asarray(
        bo, np.float32
    )
    out += corr[None, None, :]
    return out


def kernel(q, k, v, wq, bq, wk, bk, wv, bv, wo, bo) -> np.ndarray:
    nc = _get_nc()
    in_maps = make_in_maps(q, k, v, wq, bq, wk, bk, wv, bv, wo, bo)
    res = run_bass_kernel_spmd(nc, in_maps, list(range(NCORES))).results
    return assemble(res, bv, bo, wo)


# revision 27
# speedup vs baseline: 1.2357x; 1.0260x over previous
"""Multi-head attention (B=2, S=2048, D=1024, H=16, dk=64) on 8 trn2 cores.

Sharding: core c handles batch b=c//4 and 4 heads g=c%4 (heads 4g..4g+3).
Each core computes its heads' Q/K/V projections, attention, and a partial
output projection; the host sums the 4 partials per batch.

Per-core kernel layout:
  - host pre-transposes x -> xT [D, S] so projections contract over D.
  - all three x tensors stream up-front (3 SBUF buffers) with chunk DMAs
    fanned across the gpsimd/SP/Activation queues; projections chase the
    arriving chunks.
  - qhT/khT [128 = 2 heads x 64 d, S] (bf16), per head-pair hp.
  - scores are computed directly TRANSPOSED: scT[k 128-chunk, q 512] =
    (khT chunk).T @ qhT — the exp output feeds the PV matmul with no
    transpose. Two k-chunks share a [128,1024] PSUM tile; one ScalarE
    exp converts both to bf16 SBUF.
  - softmax has no max-subtraction (scores ~ N(0,1), safe in fp32/bf16).
    The denominator is free: vh gets a 65th all-ones column, so the PV
    matmul ctx_aug[65, q] = vh_aug.T @ expT accumulates ctx rows 0..63
    and the exp row-sum in row 64.
  - attention is software-pipelined per head: each head's 16 score
    matmuls + 8 exps interleave with the PREVIOUS head's 16 PV matmuls
    and one output-projection S-tile, so ScalarE (the pacing engine)
    never starves.
  - normalization happens on the tiny ctx tile, all lane-aligned: the
    denom row is broadcast over partitions on GpSimd (software op), the
    reciprocal runs on the broadcast [64,512] tile on DVE, and one fused
    multiply-cast writes ctxT (bf16). Odd heads reach partitions 64..127
    via an SBUF staging tile + DMA (engines cannot cross partitions).
  - bk/bv/bo never touch the device: bk only shifts scores by a per-row
    constant (softmax-invariant), bv/bo fold into a host-side correction
    bv @ wo.T + bo added once per batch. bq is added on-device (scaled by
    1/sqrt(dk), which is folded into wq/bq on the host).
"""

import sys

for _p in ("/opt/trn_rl_repo",):
    if _p not in sys.path:
        sys.path.insert(0, _p)

from contextlib import ExitStack

import ml_dtypes
import numpy as np

import concourse.bass as bass
import concourse.bacc as bacc_mod
import concourse.mybir as mybir
import concourse.tile as tile
from concourse.bass_utils import run_bass_kernel_spmd

BF16 = mybir.dt.bfloat16
F32 = mybir.dt.float32
AF = mybir.ActivationFunctionType

B, S, D = 2, 2048, 1024
NCORES = 8
HLOC = 4          # heads per core
DK = 64
HD = HLOC * DK    # local head dims = 256
KT_D = D // 128   # 8 k-tiles over the model dim
NST = S // 128    # 16 tiles over sequence
NQG = 4           # q groups of 512
NKC = S // 128    # 16 k chunks of 128 for scores/PV


def build_nc() -> bass.Bass:
    nc = bacc_mod.Bacc()

    xqT = nc.dram_tensor("xqT", [D, S], BF16, kind="ExternalInput")
    xkT = nc.dram_tensor("xkT", [D, S], BF16, kind="ExternalInput")
    xvT = nc.dram_tensor("xvT", [D, S], BF16, kind="ExternalInput")
    wqT = nc.dram_tensor("wqT", [D, HD], BF16, kind="ExternalInput")
    wkT = nc.dram_tensor("wkT", [D, HD], BF16, kind="ExternalInput")
    wvT = nc.dram_tensor("wvT", [D, HD], BF16, kind="ExternalInput")
    woT = nc.dram_tensor("woT", [HD, D], BF16, kind="ExternalInput")
    bqd = nc.dram_tensor("bq", [HD], BF16, kind="ExternalInput")
    out = nc.dram_tensor("out_partial", [S, D], F32, kind="ExternalOutput")

    with tile.TileContext(nc) as tc, ExitStack() as ctx:
        const = ctx.enter_context(tc.tile_pool(name="const", bufs=1))
        persist = ctx.enter_context(tc.tile_pool(name="persist", bufs=1))

        # Weights / bias resident in SBUF. wq/bq/wk load first (needed
        # earliest); wv/wo are triggered after the x loads.
        wq_s = const.tile([128, KT_D, HD], BF16, tag="wq")
        wk_s = const.tile([128, KT_D, HD], BF16, tag="wk")
        wv_s = const.tile([128, KT_D, HD], BF16, tag="wv")
        wo_s = const.tile([128, 2, D], BF16, tag="wo")
        bq_s = const.tile([1, HD], BF16, tag="bq")
        ones_row = const.tile([1, S], BF16, tag="ones")
        nc.vector.memset(ones_row, 1.0)
        # bf16 ones column used by the PE rank-1 broadcast of the softmax
        # reciprocal row (row 64 = the lane the denominator lives on)
        ones_col = const.tile([128, DK], BF16, tag="onesc")
        nc.vector.memset(ones_col, 1.0)
        nc.sync.dma_start(wq_s, wqT.rearrange("(t p) n -> p t n", p=128))
        nc.sync.dma_start(bq_s, bqd.rearrange("(o n) -> o n", o=1))
        nc.sync.dma_start(wk_s, wkT.rearrange("(t p) n -> p t n", p=128))

        # Projection outputs resident in SBUF.
        qhT = [persist.tile([128, S], BF16, tag=f"qhT{m}", name=f"qhT{m}") for m in range(2)]
        khT = [persist.tile([128, S], BF16, tag=f"khT{m}", name=f"khT{m}") for m in range(2)]
        # vh with a 65th all-ones column per (head, k-chunk): PV's 65th
        # output row becomes the softmax denominator.
        vh_aug = persist.tile([128, HLOC, NST, DK + 1], BF16, tag="vh")
        nc.vector.memset(vh_aug[:, :, :, DK : DK + 1], 1.0)
        ctxT = [persist.tile([128, S], BF16, tag=f"ctxT{m}", name=f"ctxT{m}") for m in range(2)]

        # ---- Phase 1: projections -------------------------------------
        # x chunk DMAs fan out over the three DMA-capable queues (gpsimd
        # swdge, SP hwdge, Activation hwdge) and all three tensors load
        # up-front into 3 buffers — phase 1 is input-bandwidth-bound.
        dma_engines = [nc.gpsimd, nc.sync, nc.scalar]

        def load_x(dst, src):
            for _kt in range(KT_D):
                dma_engines[_kt % 3].dma_start(
                    dst[:, _kt], src[_kt * 128 : (_kt + 1) * 128, :]
                )

        with tc.tile_pool(name="xload", bufs=3) as xpool:
          with tc.tile_pool(name="qk_psum", bufs=2, space="PSUM") as qkp:
            xq_t = xpool.tile([128, KT_D, S], BF16, tag="x", name="xq_t")
            xk_t = xpool.tile([128, KT_D, S], BF16, tag="x", name="xk_t")
            xv_t = xpool.tile([128, KT_D, S], BF16, tag="x", name="xv_t")
            load_x(xq_t, xqT)
            load_x(xk_t, xkT)
            load_x(xv_t, xvT)
            nc.sync.dma_start(wv_s, wvT.rearrange("(t p) n -> p t n", p=128))
            nc.sync.dma_start(wo_s, woT.rearrange("(t p) n -> p t n", p=128))

            for m in range(2):
                ps = qkp.tile([128, S], F32, tag="proj")
                for kt in range(KT_D):
                    for nb in range(4):
                        nc.tensor.matmul(
                            ps[:, nb * 512 : (nb + 1) * 512],
                            lhsT=wq_s[:, kt, m * 128 : (m + 1) * 128],
                            rhs=xq_t[:, kt, nb * 512 : (nb + 1) * 512],
                            start=(kt == 0),
                            stop=False,
                        )
                # bias as rank-1 matmul: qhT[p, n] += bq[p] * 1
                for nb in range(4):
                    nc.tensor.matmul(
                        ps[:, nb * 512 : (nb + 1) * 512],
                        lhsT=bq_s[0:1, m * 128 : (m + 1) * 128],
                        rhs=ones_row[0:1, nb * 512 : (nb + 1) * 512],
                        start=False,
                        stop=True,
                    )
                nc.vector.tensor_copy(qhT[m], ps)

            for m in range(2):
                ps = qkp.tile([128, S], F32, tag="proj")
                for kt in range(KT_D):
                    for nb in range(4):
                        nc.tensor.matmul(
                            ps[:, nb * 512 : (nb + 1) * 512],
                            lhsT=wk_s[:, kt, m * 128 : (m + 1) * 128],
                            rhs=xk_t[:, kt, nb * 512 : (nb + 1) * 512],
                            start=(kt == 0),
                            stop=(kt == KT_D - 1),
                        )
                nc.vector.tensor_copy(khT[m], ps)

          with tc.tile_pool(name="v_psum", bufs=4, space="PSUM") as vps:
                for st in range(NST):
                    ps = vps.tile([128, HD], F32, tag="vproj")
                    for kt in range(KT_D):
                        nc.tensor.matmul(
                            ps,
                            lhsT=xv_t[:, kt, st * 128 : (st + 1) * 128],
                            rhs=wv_s[:, kt, :],
                            start=(kt == 0),
                            stop=(kt == KT_D - 1),
                        )
                    # scatter the 4 heads' [128, 64] blocks into the
                    # 65-strided vh_aug layout (ones column untouched)
                    nc.vector.tensor_copy(vh_aug[:, :, st, 0:DK], ps)

        # ---- Phase 2+3: software-pipelined attention + out-proj -------
        with (
            tc.tile_pool(name="sc_psum", bufs=2, space="PSUM") as scp,
            tc.tile_pool(name="ctx_psum", bufs=2, space="PSUM") as ctxp,
            tc.tile_pool(name="bc_psum", bufs=2, space="PSUM") as bcpp,
            tc.tile_pool(name="exp_pool", bufs=3) as epool,
            tc.tile_pool(name="stat_pool", bufs=2) as stp,
            tc.tile_pool(name="bc_pool", bufs=2) as bcp,
            tc.tile_pool(name="ctxs_pool", bufs=2) as ctxsp,
            tc.tile_pool(name="out_sbuf", bufs=3) as outs,
        ):
            def emit_outproj_st(st):
                ops = scp.tile([128, 1024], F32, tag="sc", name="ops")
                for nb in range(2):
                    for kt in range(2):
                        nc.tensor.matmul(
                            ops[:, nb * 512 : (nb + 1) * 512],
                            lhsT=ctxT[kt][:, st * 128 : (st + 1) * 128],
                            rhs=wo_s[:, kt, nb * 512 : (nb + 1) * 512],
                            start=(kt == 0),
                            stop=(kt == 1),
                        )
                ob = outs.tile([128, D], F32, tag="ob")
                nc.vector.tensor_copy(ob, ops)
                nc.sync.dma_start(out[st * 128 : (st + 1) * 128, :], ob)

            def emit_pv_pair(stage, kp):
                _qg, h_, exp_, ctx_ = stage[:4]
                for j in range(2):
                    kt = kp * 2 + j
                    nc.tensor.matmul(
                        ctx_[0 : DK + 1, :],
                        lhsT=vh_aug[:, h_, kt, :],
                        rhs=exp_[:, kp, j * 512 : (j + 1) * 512],
                        start=(kt == 0),
                        stop=(kt == NKC - 1),
                    )

            def emit_norm(stage):
                qg_, h_, _exp, ctx_ = stage[:4]
                hp_, e_ = divmod(h_, 2)
                q0_ = qg_ * 512
                # native reciprocal of the denom row in-lane, cast to bf16
                # in-lane, then PE rank-1 bf16 broadcast.
                den = stp.tile([128, 512], F32, tag="den", name="den")
                nc.vector.reciprocal(den[DK : DK + 1, :], ctx_[DK : DK + 1, :])
                den16 = stp.tile([128, 512], BF16, tag="den16", name="den16")
                nc.vector.tensor_copy(den16[DK : DK + 1, :], den[DK : DK + 1, :])
                bc_ps = bcpp.tile([DK, 512], F32, tag="bcp", name="bc_ps")
                nc.tensor.matmul(
                    bc_ps,
                    lhsT=ones_col[DK : DK + 1, :],
                    rhs=den16[DK : DK + 1, :],
                    start=True,
                    stop=True,
                )
                rec = bcp.tile([DK, 512], F32, tag="rec", name="rec")
                nc.vector.tensor_copy(rec, bc_ps)
                if e_ == 0:
                    nc.vector.tensor_mul(
                        ctxT[hp_][0:DK, q0_ : q0_ + 512], ctx_[0:DK, :], rec
                    )
                else:
                    ctxs = ctxsp.tile([DK, 512], BF16, tag="ctxs", name="ctxs")
                    nc.vector.tensor_mul(ctxs, ctx_[0:DK, :], rec)
                    nc.sync.dma_start(
                        ctxT[hp_][DK : 2 * DK, q0_ : q0_ + 512], ctxs
                    )

            stages = [(qg, h) for qg in range(NQG) for h in range(HLOC)]
            pending: list[int] = []
            prev = None   # stage i-1: its PV matmuls + norm run during this stage
            for qg, h in stages:
                hp, e = divmod(h, 2)
                po, q0 = e * 64, qg * 512
                exp_t = epool.tile([128, NKC // 2, 1024], BF16, tag="exp", name="exp_t")
                ctx_ps = ctxp.tile([128, 512], F32, tag="ctx", name="ctx_ps")
                for kp in range(NKC // 2):
                    sc = scp.tile([128, 1024], F32, tag="sc", name="sc")
                    for j in range(2):
                        kt = kp * 2 + j
                        nc.tensor.matmul(
                            sc[:, j * 512 : (j + 1) * 512],
                            lhsT=khT[hp][po : po + 64, kt * 128 : (kt + 1) * 128],
                            rhs=qhT[hp][po : po + 64, q0 : q0 + 512],
                            start=True,
                            stop=True,
                        )
                    nc.scalar.activation(exp_t[:, kp, :], sc, AF.Exp)
                    if prev is not None:
                        emit_pv_pair(prev, kp)
                    if kp == 4 and pending:
                        emit_outproj_st(pending.pop(0))
                    if kp == 6 and len(pending) >= 3:
                        emit_outproj_st(pending.pop(0))
                if prev is not None:
                    emit_norm(prev)
                    if prev[1] == HLOC - 1:
                        pending += [prev[0] * 4 + k for k in range(4)]
                prev = (qg, h, exp_t, ctx_ps)
            # drain: last head's PV + norm, then remaining out-proj tiles
            for kp in range(NKC // 2):
                emit_pv_pair(prev, kp)
            emit_norm(prev)
            pending += [(NQG - 1) * 4 + k for k in range(4)]
            for st in pending:
                emit_outproj_st(st)

    nc.compile()
    return nc


_CACHE: dict = {}


def _get_nc() -> bass.Bass:
    if "nc" not in _CACHE:
        _CACHE["nc"] = build_nc()
    return _CACHE["nc"]


def _bf16(x: np.ndarray) -> np.ndarray:
    return np.ascontiguousarray(x).astype(ml_dtypes.bfloat16)


def make_in_maps(q, k, v, wq, bq, wk, bk, wv, bv, wo, bo):
    scale = np.float32(1.0 / np.sqrt(DK))
    in_maps = []
    for c in range(NCORES):
        b, g = divmod(c, 4)
        hh = g * HD
        in_maps.append(
            {
                "xqT": _bf16(np.asarray(q[b], np.float32).T),
                "xkT": _bf16(np.asarray(k[b], np.float32).T),
                "xvT": _bf16(np.asarray(v[b], np.float32).T),
                "wqT": _bf16(np.asarray(wq[hh : hh + HD], np.float32).T * scale),
                "wkT": _bf16(np.asarray(wk[hh : hh + HD], np.float32).T),
                "wvT": _bf16(np.asarray(wv[hh : hh + HD], np.float32).T),
                "woT": _bf16(np.asarray(wo[:, hh : hh + HD], np.float32).T),
                "bq": _bf16(np.asarray(bq[hh : hh + HD], np.float32) * scale),
            }
        )
    return in_maps


def assemble(results, bv, bo, wo) -> np.ndarray:
    out = np.zeros((B, S, D), np.float32)
    for c in range(NCORES):
        out[c // 4] += np.asarray(results[c]["out_partial"], np.float32)
    corr = np.asarray(bv, np.float32) @ np.asarray(wo, np.float32).T + np.asarray(
        bo, np.float32
    )
    out += corr[None, None, :]
    return out


def kernel(q, k, v, wq, bq, wk, bk, wv, bv, wo, bo) -> np.ndarray:
    nc = _get_nc()
    in_maps = make_in_maps(q, k, v, wq, bq, wk, bk, wv, bv, wo, bo)
    res = run_bass_kernel_spmd(nc, in_maps, list(range(NCORES))).results
    return assemble(res, bv, bo, wo)
